# revision 25
# baseline (speedup 1.0000x reference)
"""Trainium2 Bass kernel for nn_CausalTemporalAttention (8-core data parallel).

Sharding: data-parallel over batch B across 8 cores (core i handles b=i).
Fast path (no biases, the only case the harness generates):
  Phase 1: q/k projections in fp8e4 DoubleRow (x*16, w*64 quantized on the
    host; descale folded into the exp scale), scores bf16, softmax-over-l
    chains on ACT/DVE; attention numerators E and per-head reciprocals U
    stay resident.
  Phase 2: v projection in bf16 (the layer-axis softmax makes out2 a
    ~256-term token sum that amplifies v noise; fp8 here busts the 2e-2
    rel-err budget).
  Phase 3: h=7 shortcut (attn == 1 exactly -> out2 = column-sum of v via a
    ones matmul), then per head: at = E*U in place on gpsimd, attn@v,
    GT scatter, and an s'-moving out-projection computing only the nonzero
    s' >= 32h columns; output yT [E, H*S] in bf16, host transposes and
    zero-fills.
Bias fallback: the previous (slower) bf16 kernel, kept for generality.
"""

import sys

import numpy as np
import ml_dtypes

if "/opt/trn_rl_repo" not in sys.path:
    sys.path.insert(0, "/opt/trn_rl_repo")

B, L, S, E = 8, 8, 256, 1024
H, d = 8, E // 8
T = L * S
NE = E // 128
F = 3 * E

PAIRS = [(l, h) for h in range(H) for l in range(h, L)]
BLK = {p: i for i, p in enumerate(PAIRS)}

_BUILD_CACHE = {}


def build_fast():
    import concourse.bass as bass
    import concourse.tile as tile
    import concourse.mybir as mybir
    from concourse import bacc
    from contextlib import ExitStack

    dt = mybir.dt
    AF = mybir.ActivationFunctionType
    PM = mybir.MatmulPerfMode

    nc = bacc.Bacc("TRN2", target_bir_lowering=False, debug=False, num_devices=8)

    x8_d = nc.dram_tensor("x8", [E, T], dt.float8e4, kind="ExternalInput").ap()
    xT_d = nc.dram_tensor("xT", [E, T], dt.bfloat16, kind="ExternalInput").ap()
    wqk8_d = nc.dram_tensor("wqk8", [2, 7, 128, E], dt.float8e4, kind="ExternalInput").ap()
    wv_d = nc.dram_tensor("wvT", [E, E], dt.bfloat16, kind="ExternalInput").ap()
    wo_d = nc.dram_tensor("woutT", [E, E], dt.bfloat16, kind="ExternalInput").ap()
    dec_d = nc.dram_tensor("decay", [128, L * H], dt.float32, kind="ExternalInput").ap()
    id_d = nc.dram_tensor("ident", [128, 128], dt.bfloat16, kind="ExternalInput").ap()
    y_d = nc.dram_tensor("y", [E, H * S], dt.bfloat16, kind="ExternalOutput").ap()

    with ExitStack() as ctx:
        ctx.enter_context(
            nc.allow_low_precision(
                reason="fp8 q/k path + bf16 softmax; end-to-end rel err ~1.4e-2"
            )
        )
        tc = ctx.enter_context(tile.TileContext(nc))

        consts = ctx.enter_context(tc.tile_pool(name="consts", bufs=1))
        x8_sb = [consts.tile([128, 2, T], dt.float8e4, name=f"x8_{c}") for c in range(4)]
        xT_sb = [consts.tile([128, T], dt.bfloat16, name=f"xT{e}") for e in range(NE)]
        wqk8_sb = {
            (part, h): consts.tile([128, 4, 2, 128], dt.float8e4, name=f"w8{part}{h}")
            for part in ("q", "k")
            for h in range(7)
        }
        wv_sb = [consts.tile([128, E], dt.bfloat16, name=f"wv{e}") for e in range(NE)]
        wo_sb = [consts.tile([128, E], dt.bfloat16, name=f"wo{e}") for e in range(NE)]
        dec_sb = consts.tile([128, L * H], dt.float32)
        # v blocks packed ragged: per layer l only heads h<=l exist.
        voff = [0] * (L + 1)
        for l in range(L):
            voff[l + 1] = voff[l] + 2 * 128 * (l + 1)
        v_sb = consts.tile([128, voff[L]], dt.bfloat16)

        def v_slice(l, tc2, c0, c1):
            base = voff[l] + tc2 * 128 * (l + 1)
            return v_sb[:, base + c0: base + c1]

        E_all = consts.tile([128, 35, 2, S], dt.bfloat16)
        U_all = consts.tile([128, 7, 2, S], dt.bfloat16)
        ones_sb = consts.tile([128, S], dt.bfloat16)
        id_sb = consts.tile([128, 128], dt.bfloat16)
        warm_w = consts.tile([128, 128], dt.bfloat16, name="warm_w")
        warm_x = consts.tile([128, 512], dt.bfloat16, name="warm_x")
        nc.gpsimd.memset(warm_w, 0.0)
        nc.gpsimd.memset(warm_x, 0.0)
        nc.vector.memset(ones_sb, 1.0)

        # ---- DMA issue plan.  Two HW queues (SP, ACT) at ~150 GB/s each;
        # a dma_start costs ~600ns on its issuing engine, so issue counts are
        # kept small and all issues go out up front (an engine's descriptor
        # writes would otherwise block its compute FIFO).
        # SP  (6.0MB): wqk8 h0-h1, x8 halves, wqk8 h2-h6, xT half-1, wo
        # ACT (4.2MB): dec, wv, xT half-0
        def wqk8_dma(h):
            for pi, part in ((0, "q"), (1, "k")):
                nc.sync.dma_start(out=wqk8_sb[(part, h)], in_=wqk8_d[pi, h])

        def xt_dma(eng, e, half):
            eng.dma_start(
                out=xT_sb[e][:, half * 1024:(half + 1) * 1024],
                in_=xT_d[e * 128:(e + 1) * 128, half * 1024:(half + 1) * 1024],
            )

        def x8_dma(eng, c, i, half):
            eng.dma_start(
                out=x8_sb[c][:, i, half * 1024:(half + 1) * 1024],
                in_=x8_d[(2 * c + i) * 128:(2 * c + i + 1) * 128,
                         half * 1024:(half + 1) * 1024],
            )

        for half in range(2):
            for c in range(2):
                for i in range(2):
                    x8_dma(nc.sync, c, i, half)
            if half == 0:
                wqk8_dma(1)
        for h in range(2, 7):
            wqk8_dma(h)
        for e in range(NE):
            xt_dma(nc.sync, e, 1)
        for e in range(NE):
            nc.sync.dma_start(out=wo_sb[e], in_=wo_d[e * 128:(e + 1) * 128, :])

        for pi, part in ((0, "q"), (1, "k")):
            nc.scalar.dma_start(out=wqk8_sb[(part, 0)], in_=wqk8_d[pi, 0])
        for half in range(2):
            for c in range(2, 4):
                for i in range(2):
                    x8_dma(nc.scalar, c, i, half)
        nc.scalar.dma_start(out=id_sb, in_=id_d)
        nc.scalar.dma_start(out=dec_sb, in_=dec_d)
        for e in range(NE):
            nc.scalar.dma_start(out=wv_sb[e], in_=wv_d[e * 128:(e + 1) * 128, :])
        for e in range(NE):
            xt_dma(nc.gpsimd, e, 0)

        mm_ps = ctx.enter_context(tc.tile_pool(name="mm_ps", bufs=3, space="PSUM"))
        sc_ps = ctx.enter_context(tc.tile_pool(name="sc_ps", bufs=3, space="PSUM"))
        o3_ps = ctx.enter_context(tc.tile_pool(name="o3_ps", bufs=2, space="PSUM"))

        qk_pool = ctx.enter_context(tc.tile_pool(name="qk", bufs=2))
        sm_pool = ctx.enter_context(tc.tile_pool(name="smp", bufs=2))
        gt_pool = ctx.enter_context(tc.tile_pool(name="gtp", bufs=2))
        out_pool = ctx.enter_context(tc.tile_pool(name="outp", bufs=2))

        # ---- HAM warm-up: open the PE clock gate before real work arrives.
        p_warm = mm_ps.tile([128, 512], dt.float32, tag="mm", name="p_warm")
        for i in range(14):
            nc.tensor.matmul(p_warm, lhsT=warm_w, rhs=warm_x,
                             start=(i == 0), stop=(i == 13))

        # ---- phase 1: fp8 DoubleRow q/k projections + scores + softmax ----
        def qk_pair(h, part, dst, l, nl):
            p_qk = mm_ps.tile([128, nl * S], dt.float32, tag="mm", name="p_qk")
            for c in range(4):
                nc.tensor.matmul(
                    p_qk,
                    lhsT=wqk8_sb[(part, h)][:, c],
                    rhs=x8_sb[c][:, :, l * S:(l + nl) * S],
                    start=(c == 0),
                    stop=(c == 3),
                    perf_mode=PM.DoubleRow,
                )
            src = p_qk.rearrange("p (a b) -> p a b", a=nl)
            nc.vector.tensor_copy(out=dst[:, l:l + nl, :], in_=src)

        def qk_proj_thunks(h):
            # alternating q/k pair thunks so early layers complete first
            qT = qk_pool.tile([128, L, S], dt.bfloat16, tag="qT", name="qT_sb")
            kT = qk_pool.tile([128, L, S], dt.bfloat16, tag="kT", name="kT_sb")
            thunks = []
            l = h
            while l < L:
                nl = 2 if l + 1 < L else 1
                for part, dst in (("q", qT), ("k", kT)):
                    thunks.append(
                        lambda part=part, dst=dst, l=l, nl=nl:
                        qk_pair(h, part, dst, l, nl))
                l += nl
            return (qT, kT), thunks

        qT0 = qk_pool.tile([128, L, S], dt.bfloat16, tag="qT", name="qT_sb")
        kT0 = qk_pool.tile([128, L, S], dt.bfloat16, tag="kT", name="kT_sb")
        for p in range(4):
            qk_pair(0, "q", qT0, 2 * p, 2)
            qk_pair(0, "k", kT0, 2 * p, 2)

        qk_tiles = (qT0, kT0)
        for h in range(7):
            qT_sb, kT_sb = qk_tiles
            if h + 1 < 7:
                qk_tiles, thunks = qk_proj_thunks(h + 1)
            else:
                thunks = []
            # interleave next head's DR pairs between this head's scores so
            # the ACT exp chain and the DVE psum-drain copies overlap.
            for li, l in enumerate(range(h, L)):
                blk = BLK[(l, h)]
                p_sc = sc_ps.tile([128, 2, S], dt.float32, tag="sc", name="p_sc")
                for tc2 in range(2):
                    nc.tensor.matmul(
                        p_sc[:, tc2, :],
                        lhsT=kT_sb[:, l, tc2 * 128:(tc2 + 1) * 128],
                        rhs=qT_sb[:, l, :],
                        start=True,
                        stop=True,
                    )
                idx = l * H + h
                nc.scalar.activation(
                    out=E_all[:, blk],
                    in_=p_sc,
                    func=AF.Exp,
                    scale=dec_sb[:, idx:idx + 1],
                )
                if li < len(thunks):
                    thunks[li]()
            for t in thunks[len(range(h, L)):]:
                t()
            # softmax denominator summed ON THE PE: identity-matmul psum
            # accumulation over the head's exp blocks (cheap 512-row bf16
            # matmuls; keeps the slow vector engines off the critical chain).
            p_D = o3_ps.tile([128, 2, S], dt.float32, tag="oy", name="p_D")
            for li, l in enumerate(range(h, L)):
                nc.tensor.matmul(
                    p_D,
                    lhsT=id_sb,
                    rhs=E_all[:, BLK[(l, h)]],
                    start=(li == 0),
                    stop=(l == L - 1),
                )
            U32 = sm_pool.tile([128, 2, S], dt.float32, tag="R", name="U32")
            nc.vector.reciprocal_approx_fast(out=U32, in_=p_D)
            nc.vector.tensor_copy(out=U_all[:, h], in_=U32)

        # ---- phase 2: v projection (bf16) with the at = E*U multiplies
        # interleaved on DVE (attention weights are fully formed before
        # phase 3, keeping phase 3 PE-bound).
        at_muls = [(h, l) for h in range(7) for l in range(h, L)]
        n_vgroups = sum((128 * (tt // 2 + 1) + 511) // 512 for tt in range(16))
        per_group = (len(at_muls) + n_vgroups - 1) // n_vgroups
        for tt in range(16):
            l = tt // 2
            ncols = 128 * (l + 1)
            for g in range((ncols + 511) // 512):
                n_g = min(512, ncols - 512 * g)
                p_v = mm_ps.tile([128, n_g], dt.float32, tag="mm", name="p_v")
                for e in range(NE):
                    nc.tensor.matmul(
                        p_v,
                        lhsT=xT_sb[e][:, tt * 128:(tt + 1) * 128],
                        rhs=wv_sb[e][:, 512 * g: 512 * g + n_g],
                        start=(e == 0),
                        stop=(e == NE - 1),
                    )
                nc.scalar.copy(
                    out=v_slice(l, tt % 2, 512 * g, 512 * g + n_g), in_=p_v)
                for _ in range(per_group):
                    if at_muls:
                        ah, al = at_muls.pop(0)
                        ab = BLK[(al, ah)]
                        nc.vector.tensor_mul(
                            E_all[:, ab], E_all[:, ab], U_all[:, ah])
        while at_muls:
            ah, al = at_muls.pop(0)
            ab = BLK[(al, ah)]
            nc.vector.tensor_mul(E_all[:, ab], E_all[:, ab], U_all[:, ah])

        # ---- phase 3: attention + s'-moving out-projection ----
        def emit_proj(h, gt_sb, dma_eng):
            # yT[f, s'] = woT.T @ GT_h, only the nonzero s' >= 32h columns.
            n_c = S - 32 * h
            o_sb = out_pool.tile([128, NE, S], dt.bfloat16, tag="o", name="o_sb")
            for fb in range(NE):
                p_y = o3_ps.tile([128, S], dt.float32, tag="oy", name="p_y")
                for j in range(NE):
                    nc.tensor.matmul(
                        p_y[:, :n_c],
                        lhsT=wo_sb[j][:, fb * 128:(fb + 1) * 128],
                        rhs=gt_sb[:, j, 32 * h:S],
                        start=(j == 0),
                        stop=(j == NE - 1),
                    )
                if fb % 2 == 0:
                    nc.vector.tensor_copy(out=o_sb[:, fb, :n_c], in_=p_y[:, :n_c])
                else:
                    nc.scalar.copy(out=o_sb[:, fb, :n_c], in_=p_y[:, :n_c])
            nsplit = 4 if h >= 5 else 2
            for fbh in range(nsplit):
                w_fb = NE // nsplit
                dma_eng.dma_start(
                    out=y_d[fbh * w_fb * 128:(fbh + 1) * w_fb * 128,
                            h * S + 32 * h:(h + 1) * S].rearrange(
                        "(fb p) c -> p fb c", p=128
                    ),
                    in_=o_sb[:, fbh * w_fb:(fbh + 1) * w_fb, :n_c],
                )

        # h=7 first: attn == 1 exactly (single unmasked layer) -> out2 is the
        # column sum of v[l=7, h=7]; its out-projection fills the PE at the
        # phase-3 start where head 0 has no deferred work yet.
        gt7 = gt_pool.tile([128, NE, S], dt.bfloat16, tag="gt", name="gt7_sb")
        p_o7 = sc_ps.tile([128, S], dt.float32, tag="sc", name="p_o7")
        for tc2 in range(2):
            nc.tensor.matmul(
                p_o7,
                lhsT=v_slice(7, tc2, 7 * 128, 8 * 128),
                rhs=ones_sb,
                start=(tc2 == 0),
                stop=(tc2 == 1),
            )
        nc.vector.tensor_copy(
            out=gt7[:, :, 7 * 32:8 * 32],
            in_=p_o7.rearrange("p (si j) -> p j si", j=8),
        )

        pending = (7, gt7)
        for h in range(7):
            gt_sb = gt_pool.tile([128, NE, S], dt.bfloat16, tag="gt", name="gt_sb")
            for li, l in enumerate(range(h, L)):
                blk = BLK[(l, h)]
                p_o2 = sc_ps.tile([128, S], dt.float32, tag="sc", name="p_o2")
                for tc2 in range(2):
                    nc.tensor.matmul(
                        p_o2,
                        lhsT=v_slice(l, tc2, h * 128, (h + 1) * 128),
                        rhs=E_all[:, blk, tc2, :],
                        start=(tc2 == 0),
                        stop=(tc2 == 1),
                    )
                geng = nc.vector if l % 2 == 0 else nc.scalar
                if l % 2 == 0:
                    nc.vector.tensor_copy(
                        out=gt_sb[:, :, l * 32:(l + 1) * 32],
                        in_=p_o2.rearrange("p (si j) -> p j si", j=8),
                    )
                else:
                    nc.scalar.copy(
                        out=gt_sb[:, :, l * 32:(l + 1) * 32],
                        in_=p_o2.rearrange("p (si j) -> p j si", j=8),
                    )
                if li == 1 and pending is not None:
                    emit_proj(pending[0], pending[1], nc.sync)
                    pending = None
            pending = (h, gt_sb)
        emit_proj(pending[0], pending[1], nc.scalar)

    nc.compile()
    return nc


def prepare_in_maps_fast(x, w_qkv, w_out, decay_params):
    bf16 = ml_dtypes.bfloat16
    e4 = ml_dtypes.float8_e4m3fn

    wqk = w_qkv[:2 * E]                                          # [2E, E]
    wqk8 = np.ascontiguousarray(
        (wqk.astype(np.float32) * 64.0)
        .reshape(2, H, d, NE, 128)
        .transpose(0, 1, 4, 3, 2)
    ).reshape(2, H, 128, E)[:, :7].astype(e4)
    wqk8 = np.ascontiguousarray(wqk8)
    wvT = np.ascontiguousarray(w_qkv[2 * E:].astype(bf16).T)     # [E, E]
    woutT = np.ascontiguousarray(w_out.astype(bf16).T)           # [E, E]

    ident = np.eye(128, dtype=ml_dtypes.bfloat16)
    in_maps = []
    for b in range(B):
        xTf = x[b].reshape(T, E).T                               # [E, T] fp32
        xT = np.ascontiguousarray(xTf.astype(bf16))
        x8 = np.ascontiguousarray((xTf * 16.0).astype(e4))
        dec = np.ascontiguousarray(
            np.broadcast_to(
                (decay_params[b, :L, :H] / (np.sqrt(d) * (16.0 * 64.0) ** 2))
                .astype(np.float32)
                .reshape(1, L * H),
                (128, L * H),
            )
        )
        in_maps.append({
            "x8": x8, "xT": xT, "wqk8": wqk8, "wvT": wvT,
            "woutT": woutT, "decay": dec, "ident": ident,
        })
    return in_maps


def unshard_fast(results):
    out = np.empty((B, H, S, E), dtype=np.float32)
    for b, r in enumerate(results):
        yT = np.asarray(r["y"]).astype(np.float32)               # [E, H*S]
        arr = yT.reshape(E, H, S).transpose(1, 2, 0).copy()      # [H, S, E]
        for h in range(H):
            arr[h, :32 * h, :] = 0.0
        out[b] = arr
    return out


def _build_bias(with_bias):
    import concourse.bass as bass
    import concourse.tile as tile
    import concourse.mybir as mybir
    from concourse import bacc
    from contextlib import ExitStack

    dt = mybir.dt
    AF = mybir.ActivationFunctionType

    nc = bacc.Bacc("TRN2", target_bir_lowering=False, debug=False, num_devices=8)

    xT_d = nc.dram_tensor("xT", [E, T], dt.bfloat16, kind="ExternalInput").ap()
    # q/k weights pre-packed on host as [part, head, p, e*128+m] so each
    # (part, head) loads with ONE contiguous-per-partition DMA.
    wqkp_d = nc.dram_tensor("wqkp", [2, H, 128, E], dt.bfloat16, kind="ExternalInput").ap()
    wv_d = nc.dram_tensor("wvT", [E, E], dt.bfloat16, kind="ExternalInput").ap()
    wo_d = nc.dram_tensor("woutT", [E, E], dt.bfloat16, kind="ExternalInput").ap()
    dec_d = nc.dram_tensor("decay", [128, L * H], dt.float32, kind="ExternalInput").ap()
    id_d = nc.dram_tensor("ident", [128, 128], dt.bfloat16, kind="ExternalInput").ap()
    if with_bias:
        bq_d = nc.dram_tensor("bqkv", [1, F], dt.bfloat16, kind="ExternalInput").ap()
        bo_d = nc.dram_tensor("bout", [1, E], dt.bfloat16, kind="ExternalInput").ap()
        bor_d = nc.dram_tensor("bout_row", [128, E], dt.float32, kind="ExternalInput").ap()
    y_d = nc.dram_tensor("y", [H, S, E], dt.float32, kind="ExternalOutput").ap()

    with ExitStack() as ctx:
        ctx.enter_context(
            nc.allow_low_precision(
                reason="bf16 softmax intermediates; end-to-end error ~3e-3 of scale"
            )
        )
        tc = ctx.enter_context(tile.TileContext(nc))

        consts = ctx.enter_context(tc.tile_pool(name="consts", bufs=1))
        # One tile + one DMA writer per chunk: a tile written by DMAs spread
        # over many HW queues gives its first consumer more sync waits than
        # the MM instruction can encode ("Too many sync wait commands"), and
        # fine-grained tiles let compute start as soon as its chunk lands.
        xT_sb = [consts.tile([128, T], dt.bfloat16, name=f"xT{e}") for e in range(NE)]
        wqv_sb = [consts.tile([128, E], dt.bfloat16, name=f"wqv{e}") for e in range(NE)]
        # q/k weights: one tile per (part, head) holding all e-chunks
        # side-by-side, loaded head-major so head 0's tiles land first.
        wqk_sb = {
            (part, h): consts.tile([128, E], dt.bfloat16, name=f"w{part}{h}")
            for part in ("q", "k")
            for h in range(H)
        }
        wo_sb = [consts.tile([128, E], dt.bfloat16, name=f"wo{e}") for e in range(NE)]
        dec_sb = consts.tile([128, L * H], dt.float32)
        v_sb = consts.tile([128, len(PAIRS), 2, d], dt.bfloat16)
        zrow_sb = consts.tile([128, 512], dt.float32)

        if with_bias:
            bq_sb = consts.tile([1, F], dt.bfloat16)
            bo_sb = consts.tile([1, E], dt.bfloat16)
            ones_sb = consts.tile([1, 512], dt.bfloat16)
            borow_sb = consts.tile([128, E], dt.float32)
            nc.sync.dma_start(out=bq_sb, in_=bq_d)
            nc.sync.dma_start(out=bo_sb, in_=bo_d)
            nc.sync.dma_start(out=borow_sb, in_=bor_d)
            nc.vector.memset(ones_sb, 1.0)

        nc.vector.memset(zrow_sb, 0.0)
        nc.sync.dma_start(out=dec_sb, in_=dec_d)
        # Three DMA queues in parallel, each loading in consumption order:
        #   SP:    xT chunks (feed everything, needed from the first MM)
        #   ACT:   q/k weight tiles, head-major (head 0 first)
        #   Pool:  v weights (needed ~15us in), then out-proj weights (~60us)
        # Each dma_start lands on ONE ~25-40GB/s HW ring (8 rings per issuing
        # engine). Split tensors into chunks and issue in exact consumption
        # order, ~6MB per issuing queue:
        #   SP:  xT quarter-waves (quarter q == layer-pair q), then w_out
        #   ACT: q/k weights for heads 0-1, then w_v, then remaining heads
        def _wqk_dma(part, pi, h):
            for half in range(2):
                nc.scalar.dma_start(
                    out=wqk_sb[(part, h)][:, half * 512:(half + 1) * 512],
                    in_=wqkp_d[pi, h, :, half * 512:(half + 1) * 512],
                )

        def _wqk_dma_sp(part, pi, h):
            for half in range(2):
                nc.sync.dma_start(
                    out=wqk_sb[(part, h)][:, half * 512:(half + 1) * 512],
                    in_=wqkp_d[pi, h, :, half * 512:(half + 1) * 512],
                )

        # SP row (fast): head-0 weights (first-MM gate), xT quarters, head-1
        # weights, w_out. ACT row (slower): v weights, then heads 2..7.
        for pi, part in ((0, "q"), (1, "k")):
            _wqk_dma_sp(part, pi, 0)
        for q in range(4):
            if q == 3:
                # head-1 weights land before xT's last quarter so qk_proj(1)
                # never stalls at the front->head-loop transition
                for pi, part in ((0, "q"), (1, "k")):
                    _wqk_dma_sp(part, pi, 1)
            for e in range(NE):
                nc.sync.dma_start(
                    out=xT_sb[e][:, q * 512:(q + 1) * 512],
                    in_=xT_d[e * 128:(e + 1) * 128, q * 512:(q + 1) * 512],
                )
        for e in range(NE):
            for half in range(2):
                nc.sync.dma_start(
                    out=wo_sb[e][:, half * 512:(half + 1) * 512],
                    in_=wo_d[e * 128:(e + 1) * 128, half * 512:(half + 1) * 512],
                )
        for g in range(2):
            for e in range(NE):
                nc.scalar.dma_start(
                    out=wqv_sb[e][:, g * 512:(g + 1) * 512],
                    in_=wv_d[e * 128:(e + 1) * 128, g * 512:(g + 1) * 512],
                )
        for h in range(2, H):
            for pi, part in ((0, "q"), (1, "k")):
                _wqk_dma(part, pi, h)

        mm_ps = ctx.enter_context(tc.tile_pool(name="mm_ps", bufs=3, space="PSUM"))
        sc_ps = ctx.enter_context(tc.tile_pool(name="sc_ps", bufs=3, space="PSUM"))
        o2_ps = ctx.enter_context(tc.tile_pool(name="o2_ps", bufs=2, space="PSUM"))

        qk_pool = ctx.enter_context(tc.tile_pool(name="qk", bufs=2))
        exp_pool = ctx.enter_context(tc.tile_pool(name="expp", bufs=1))
        sm_pool = ctx.enter_context(tc.tile_pool(name="smp", bufs=2))
        at_pool = ctx.enter_context(tc.tile_pool(name="atp", bufs=3))
        gt_pool = ctx.enter_context(tc.tile_pool(name="gtp", bufs=2))
        out_pool = ctx.enter_context(tc.tile_pool(name="outp", bufs=3))

        def v_proj(tts, gs=(0, 1)):
            # v projection (natural [token, dd] layout): stationary xT tile,
            # moving w columns. Only heads h <= l are ever read.
            for tt in tts:
                l = tt // 2
                ncols = 128 * (l + 1)
                for g in range((ncols + 511) // 512):
                    if g not in gs:
                        continue
                    n_g = min(512, ncols - 512 * g)
                    p_v = mm_ps.tile([128, n_g], dt.float32, tag="mm", name="p_v")
                    for e in range(NE):
                        nc.tensor.matmul(
                            p_v,
                            lhsT=xT_sb[e][:, tt * 128:(tt + 1) * 128],
                            rhs=wqv_sb[e][:, 512 * g: 512 * g + n_g],
                            start=(e == 0),
                            stop=(e == NE - 1) and not with_bias,
                        )
                    if with_bias:
                        nc.tensor.matmul(
                            p_v,
                            lhsT=ones_sb[:, :128],
                            rhs=bq_sb[:, 2 * E + 512 * g: 2 * E + 512 * g + n_g],
                            start=False,
                            stop=True,
                        )
                    for hh in range(4 * g, 4 * g + n_g // 128):
                        nc.vector.tensor_copy(
                            out=v_sb[:, BLK[(l, hh)], tt % 2, :],
                            in_=p_v[:, (hh - 4 * g) * 128:(hh - 4 * g + 1) * 128],
                        )

        # ---- per-head pipeline: q/k projection -> scores -> softmax-over-l ->
        # attn@v -> scatter into the scrambled proj input -> out projection.
        # The q/k projection for head h+1 is emitted between head h's scores
        # and attn@v so the PE has work while the softmax chain (ACT+DVE) runs.
        def qk_pair(h, part, base, dst, l, nl):
            p_qk = mm_ps.tile([128, nl * S], dt.float32, tag="mm", name="p_qk")
            for e in range(NE):
                nc.tensor.matmul(
                    p_qk,
                    lhsT=wqk_sb[(part, h)][:, e * 128:(e + 1) * 128],
                    rhs=xT_sb[e][:, l * S:(l + nl) * S],
                    start=(e == 0),
                    stop=(e == NE - 1) and not with_bias,
                )
            if with_bias:
                nc.tensor.matmul(
                    p_qk,
                    lhsT=bq_sb[:, base + h * 128: base + (h + 1) * 128],
                    rhs=ones_sb[:, :nl * S],
                    start=False,
                    stop=True,
                )
            src = p_qk.rearrange("p (a b) -> p a b", a=nl)
            nc.vector.tensor_copy(out=dst[:, l:l + nl, :], in_=src)

        def qk_proj(h):
            qT = qk_pool.tile([128, L, S], dt.bfloat16, tag="qT", name="qT_sb")
            kT = qk_pool.tile([128, L, S], dt.bfloat16, tag="kT", name="kT_sb")
            for part, base, dst in (("q", 0, qT), ("k", E, kT)):
                l = h
                while l < L:
                    nl = 2 if l + 1 < L else 1  # pair layers: N=512 moving dim
                    qk_pair(h, part, base, dst, l, nl)
                    l += nl
            return qT, kT

        # ---- HAM warm-up: dummy matmuls with no DMA deps fill the input
        # lead-in and open the PE clock gate before real work arrives.
        warm_w = consts.tile([128, 128], dt.bfloat16, name="warm_w")
        warm_x = consts.tile([128, 512], dt.bfloat16, name="warm_x")
        nc.vector.memset(warm_w, 0.0)
        nc.vector.memset(warm_x, 0.0)
        p_warm = mm_ps.tile([128, 512], dt.float32, tag="mm", name="p_warm")
        for i in range(14):
            nc.tensor.matmul(p_warm, lhsT=warm_w, rhs=warm_x,
                             start=(i == 0), stop=(i == 13))

        # ---- front: head-0 q/k pairs and v-groups interleaved by xT quarter,
        # matching DMA arrival order so the in-order PE never blocks on a
        # not-yet-loaded chunk.
        qT0 = qk_pool.tile([128, L, S], dt.bfloat16, tag="qT", name="qT_sb")
        kT0 = qk_pool.tile([128, L, S], dt.bfloat16, tag="kT", name="kT_sb")
        for p in range(4):
            qk_pair(0, "q", 0, qT0, 2 * p, 2)
            qk_pair(0, "k", E, kT0, 2 * p, 2)
            v_proj(range(4 * p, 4 * p + 4))
        def emit_proj(h, gt_sb):
            # out projection for head h: y[h, s', :] = GT.T @ woutT (+ b_out)
            for st in range(2):
                if st == 0 and h >= 4:
                    # rows s' in [0,128) are exactly zero for h >= 4
                    for ng in range(2):
                        src = borow_sb[:, ng * 512:(ng + 1) * 512] if with_bias else zrow_sb
                        nc.sync.dma_start(
                            out=y_d[h, :128, ng * 512:(ng + 1) * 512], in_=src
                        )
                    continue
                for ng in range(2):
                    p_pr = mm_ps.tile([128, 512], dt.float32, tag="mm", name="p_pr")
                    for j in range(NE):
                        nc.tensor.matmul(
                            p_pr,
                            lhsT=gt_sb[:, j, st * 128:(st + 1) * 128],
                            rhs=wo_sb[j][:, ng * 512:(ng + 1) * 512],
                            start=(j == 0),
                            stop=(j == NE - 1) and not with_bias,
                        )
                    if with_bias:
                        nc.tensor.matmul(
                            p_pr,
                            lhsT=ones_sb[:, :128],
                            rhs=bo_sb[:, ng * 512:(ng + 1) * 512],
                            start=False,
                            stop=True,
                        )
                    o_sb = out_pool.tile([128, 512], dt.float32, tag="o", name="o_sb")
                    nc.scalar.copy(out=o_sb, in_=p_pr)
                    nc.sync.dma_start(
                        out=y_d[h, st * 128:(st + 1) * 128, ng * 512:(ng + 1) * 512],
                        in_=o_sb,
                    )

        qk_tiles = (qT0, kT0)
        pending_proj = None  # (h, gt) deferred one head: extra PE gap filler
        for h in range(H):
            qT_sb, kT_sb = qk_tiles
            # scores (transposed [t, s]) + exp with decay/sqrt(d) folded into
            # the activation scale; D accumulates the softmax denominator.
            # Softmax intermediates in bf16: DVE runs 2-byte SBUF ops in the
            # fast perf modes, and the end-to-end error stays ~3e-3 of scale.
            E_sb = exp_pool.tile([128, L, 2, S], dt.bfloat16, tag="E", name="E_sb")
            D_sb = sm_pool.tile([128, 2, S], dt.bfloat16, tag="D", name="D_sb")
            for l in range(h, L):
                p_sc = sc_ps.tile([128, 2, S], dt.float32, tag="sc", name="p_sc")
                for tc2 in range(2):
                    nc.tensor.matmul(
                        p_sc[:, tc2, :],
                        lhsT=kT_sb[:, l, tc2 * 128:(tc2 + 1) * 128],
                        rhs=qT_sb[:, l, :],
                        start=True,
                        stop=True,
                    )
                idx = l * H + h
                nc.scalar.activation(
                    out=E_sb[:, l, :, :],
                    in_=p_sc,
                    func=AF.Exp,
                    scale=dec_sb[:, idx:idx + 1],
                )
                if l == h:
                    nc.vector.tensor_copy(out=D_sb, in_=E_sb[:, l, :, :])
                else:
                    nc.vector.tensor_add(D_sb, D_sb, E_sb[:, l, :, :])

            # next head's projection + previous head's out-projection fill
            # the PE while this head's softmax chain finishes on ACT/DVE
            if h + 1 < H:
                qk_tiles = qk_proj(h + 1)
            if pending_proj is not None:
                emit_proj(*pending_proj)
                pending_proj = None

            U_sb = sm_pool.tile([128, 2, S], dt.bfloat16, tag="U", name="U_sb")
            nc.vector.reciprocal(out=U_sb, in_=D_sb)

            # attn @ v (output transposed [dd, s]) and scatter into GT, the
            # transposed input of the out-projection:
            #   GT[dd, j, l*32 + si] = out2T[dd, si*8 + j]
            gt_sb = gt_pool.tile([128, L, S], dt.bfloat16, tag="gt", name="gt_sb")
            if h > 0:
                nc.vector.memset(gt_sb[:, :, :h * 32], 0.0)
            for l in range(h, L):
                at_sb = at_pool.tile([128, 2, S], dt.bfloat16, tag="at", name="at_sb")
                nc.vector.tensor_mul(at_sb, E_sb[:, l, :, :], U_sb)
                p_o2 = o2_ps.tile([128, S], dt.float32, tag="o2", name="p_o2")
                for tc2 in range(2):
                    nc.tensor.matmul(
                        p_o2,
                        lhsT=v_sb[:, BLK[(l, h)], tc2, :],
                        rhs=at_sb[:, tc2, :],
                        start=(tc2 == 0),
                        stop=(tc2 == 1),
                    )
                geng = nc.vector if l % 2 == 0 else nc.scalar
                if l % 2 == 0:
                    nc.vector.tensor_copy(
                        out=gt_sb[:, :, l * 32:(l + 1) * 32],
                        in_=p_o2.rearrange("p (si j) -> p j si", j=8),
                    )
                else:
                    nc.scalar.copy(
                        out=gt_sb[:, :, l * 32:(l + 1) * 32],
                        in_=p_o2.rearrange("p (si j) -> p j si", j=8),
                    )

            pending_proj = (h, gt_sb)
        emit_proj(*pending_proj)

    nc.compile()
    return nc


def _prepare_in_maps(x, w_qkv, b_qkv, w_out, b_out, decay_params):
    bf16 = ml_dtypes.bfloat16
    with_bias = bool(np.any(b_qkv != 0) or np.any(b_out != 0))

    wqk_bf = w_qkv[:2 * E].astype(bf16)                          # [2E, E]
    # [part, head, m, e, p] -> [part, head, p, e, m]: each (part, head) tile
    # is the stationary lhsT for all e-chunks, contiguous in DRAM.
    wqkp = np.ascontiguousarray(
        wqk_bf.reshape(2, H, d, NE, 128).transpose(0, 1, 4, 3, 2)
    ).reshape(2, H, 128, E)
    wvT = np.ascontiguousarray(w_qkv[2 * E:].astype(bf16).T)     # [E, E]
    woutT = np.ascontiguousarray(w_out.astype(bf16).T)           # [E, E]

    ident = np.eye(128, dtype=ml_dtypes.bfloat16)
    in_maps = []
    for b in range(B):
        xT = np.ascontiguousarray(
            x[b].reshape(T, E).astype(bf16).T                    # [E, T]
        )
        dec = np.ascontiguousarray(
            np.broadcast_to(
                (decay_params[b, :L, :H] / np.float32(np.sqrt(d)))
                .astype(np.float32)
                .reshape(1, L * H),
                (128, L * H),
            )
        )
        m = {"xT": xT, "wqkp": wqkp, "wvT": wvT, "woutT": woutT, "decay": dec}
        if with_bias:
            m["bqkv"] = np.ascontiguousarray(b_qkv.astype(bf16).reshape(1, F))
            m["bout"] = np.ascontiguousarray(b_out.astype(bf16).reshape(1, E))
            m["bout_row"] = np.ascontiguousarray(
                np.broadcast_to(b_out.astype(np.float32).reshape(1, E), (128, E))
            )
        in_maps.append(m)
    return with_bias, in_maps

def _get_nc(key):
    if key not in _BUILD_CACHE:
        if key == "fast":
            _BUILD_CACHE[key] = build_fast()
        else:
            _BUILD_CACHE[key] = _build_bias(True)
    return _BUILD_CACHE[key]


def _run(x, w_qkv, b_qkv, w_out, b_out, decay_params, **spmd_kwargs):
    from concourse.bass_utils import run_bass_kernel_spmd

    with_bias = bool(np.any(b_qkv != 0) or np.any(b_out != 0))
    if with_bias:
        _, in_maps = _prepare_in_maps(x, w_qkv, b_qkv, w_out, b_out, decay_params)
        nc = _get_nc("bias")
        res = run_bass_kernel_spmd(nc, in_maps, core_ids=list(range(B)), **spmd_kwargs)
        out = np.stack([np.asarray(r["y"]) for r in res.results], axis=0)
        return out.astype(np.float32, copy=False), res

    in_maps = prepare_in_maps_fast(x, w_qkv, w_out, decay_params)
    nc = _get_nc("fast")
    res = run_bass_kernel_spmd(nc, in_maps, core_ids=list(range(B)), **spmd_kwargs)
    out = unshard_fast(res.results)
    return out, res


def kernel(x, w_qkv, b_qkv, w_out, b_out, decay_params):
    out, _ = _run(
        np.asarray(x), np.asarray(w_qkv), np.asarray(b_qkv),
        np.asarray(w_out), np.asarray(b_out), np.asarray(decay_params),
    )
    return out


# revision 26
# speedup vs baseline: 1.0041x; 1.0041x over previous
"""Trainium2 Bass kernel for nn_CausalTemporalAttention (8-core data parallel).

Sharding: data-parallel over batch B across 8 cores (core i handles b=i).
Fast path (no biases, the only case the harness generates):
  Phase 1: q/k projections in fp8e4 DoubleRow (x*16, w*64 quantized on the
    host; descale folded into the exp scale), scores bf16, softmax-over-l
    chains on ACT/DVE; attention numerators E and per-head reciprocals U
    stay resident.
  Phase 2: v projection in bf16 (the layer-axis softmax makes out2 a
    ~256-term token sum that amplifies v noise; fp8 here busts the 2e-2
    rel-err budget).
  Phase 3: h=7 shortcut (attn == 1 exactly -> out2 = column-sum of v via a
    ones matmul), then per head: at = E*U in place on gpsimd, attn@v,
    GT scatter, and an s'-moving out-projection computing only the nonzero
    s' >= 32h columns; output yT [E, H*S] in bf16, host transposes and
    zero-fills.
Bias fallback: the previous (slower) bf16 kernel, kept for generality.
"""

import sys

import numpy as np
import ml_dtypes

if "/opt/trn_rl_repo" not in sys.path:
    sys.path.insert(0, "/opt/trn_rl_repo")

B, L, S, E = 8, 8, 256, 1024
H, d = 8, E // 8
T = L * S
NE = E // 128
F = 3 * E

PAIRS = [(l, h) for h in range(H) for l in range(h, L)]
BLK = {p: i for i, p in enumerate(PAIRS)}

_BUILD_CACHE = {}


def build_fast():
    import concourse.bass as bass
    import concourse.tile as tile
    import concourse.mybir as mybir
    from concourse import bacc
    from contextlib import ExitStack

    dt = mybir.dt
    AF = mybir.ActivationFunctionType
    PM = mybir.MatmulPerfMode

    nc = bacc.Bacc("TRN2", target_bir_lowering=False, debug=False, num_devices=8)

    x8_d = nc.dram_tensor("x8", [E, T], dt.float8e4, kind="ExternalInput").ap()
    xT_d = nc.dram_tensor("xT", [E, T], dt.bfloat16, kind="ExternalInput").ap()
    wqk8_d = nc.dram_tensor("wqk8", [2, 7, 128, E], dt.float8e4, kind="ExternalInput").ap()
    wv_d = nc.dram_tensor("wvT", [E, E], dt.bfloat16, kind="ExternalInput").ap()
    wo_d = nc.dram_tensor("woutT", [E, E], dt.bfloat16, kind="ExternalInput").ap()
    dec_d = nc.dram_tensor("decay", [128, L * H], dt.float32, kind="ExternalInput").ap()
    id_d = nc.dram_tensor("ident", [128, 128], dt.bfloat16, kind="ExternalInput").ap()
    y_d = nc.dram_tensor("y", [E, H * S], dt.bfloat16, kind="ExternalOutput").ap()

    with ExitStack() as ctx:
        ctx.enter_context(
            nc.allow_low_precision(
                reason="fp8 q/k path + bf16 softmax; end-to-end rel err ~1.4e-2"
            )
        )
        tc = ctx.enter_context(tile.TileContext(nc))

        consts = ctx.enter_context(tc.tile_pool(name="consts", bufs=1))
        x8_sb = [consts.tile([128, 2, T], dt.float8e4, name=f"x8_{c}") for c in range(4)]
        xT_sb = [consts.tile([128, T], dt.bfloat16, name=f"xT{e}") for e in range(NE)]
        wqk8_sb = {
            (part, h): consts.tile([128, 4, 2, 128], dt.float8e4, name=f"w8{part}{h}")
            for part in ("q", "k")
            for h in range(7)
        }
        wv_sb = [consts.tile([128, E], dt.bfloat16, name=f"wv{e}") for e in range(NE)]
        wo_sb = [consts.tile([128, E], dt.bfloat16, name=f"wo{e}") for e in range(NE)]
        dec_sb = consts.tile([128, L * H], dt.float32)
        # v blocks packed ragged: per layer l only heads h<=l exist.
        voff = [0] * (L + 1)
        for l in range(L):
            voff[l + 1] = voff[l] + 2 * 128 * (l + 1)
        v_sb = consts.tile([128, voff[L]], dt.bfloat16)

        def v_slice(l, tc2, c0, c1):
            base = voff[l] + tc2 * 128 * (l + 1)
            return v_sb[:, base + c0: base + c1]

        E_all = consts.tile([128, 35, 2, S], dt.bfloat16)
        U_all = consts.tile([128, 7, 2, S], dt.bfloat16)
        ones_sb = consts.tile([128, S], dt.bfloat16)
        id_sb = consts.tile([128, 128], dt.bfloat16)
        warm_w = consts.tile([128, 128], dt.bfloat16, name="warm_w")
        warm_x = consts.tile([128, 512], dt.bfloat16, name="warm_x")
        nc.gpsimd.memset(warm_w, 0.0)
        nc.gpsimd.memset(warm_x, 0.0)
        nc.vector.memset(ones_sb, 1.0)

        # ---- DMA issue plan.  Two HW queues (SP, ACT) at ~150 GB/s each;
        # a dma_start costs ~600ns on its issuing engine, so issue counts are
        # kept small and all issues go out up front (an engine's descriptor
        # writes would otherwise block its compute FIFO).
        # SP  (6.0MB): wqk8 h0-h1, x8 halves, wqk8 h2-h6, xT half-1, wo
        # ACT (4.2MB): dec, wv, xT half-0
        def wqk8_dma(h):
            for pi, part in ((0, "q"), (1, "k")):
                nc.sync.dma_start(out=wqk8_sb[(part, h)], in_=wqk8_d[pi, h])

        def xt_dma(eng, e, half):
            eng.dma_start(
                out=xT_sb[e][:, half * 1024:(half + 1) * 1024],
                in_=xT_d[e * 128:(e + 1) * 128, half * 1024:(half + 1) * 1024],
            )

        def x8_dma(eng, c, i, half):
            eng.dma_start(
                out=x8_sb[c][:, i, half * 1024:(half + 1) * 1024],
                in_=x8_d[(2 * c + i) * 128:(2 * c + i + 1) * 128,
                         half * 1024:(half + 1) * 1024],
            )

        for half in range(2):
            for c in range(2):
                for i in range(2):
                    x8_dma(nc.sync, c, i, half)
            if half == 0:
                wqk8_dma(1)
        for h in range(2, 7):
            wqk8_dma(h)
        for e in range(NE):
            xt_dma(nc.sync, e, 1)
        for e in range(NE):
            nc.sync.dma_start(out=wo_sb[e], in_=wo_d[e * 128:(e + 1) * 128, :])

        for pi, part in ((0, "q"), (1, "k")):
            nc.scalar.dma_start(out=wqk8_sb[(part, 0)], in_=wqk8_d[pi, 0])
        for half in range(2):
            for c in range(2, 4):
                for i in range(2):
                    x8_dma(nc.scalar, c, i, half)
        nc.scalar.dma_start(out=id_sb, in_=id_d)
        nc.scalar.dma_start(out=dec_sb, in_=dec_d)
        for e in range(NE):
            nc.scalar.dma_start(out=wv_sb[e], in_=wv_d[e * 128:(e + 1) * 128, :])
        for e in range(NE):
            xt_dma(nc.gpsimd, e, 0)

        mm_ps = ctx.enter_context(tc.tile_pool(name="mm_ps", bufs=3, space="PSUM"))
        sc_ps = ctx.enter_context(tc.tile_pool(name="sc_ps", bufs=3, space="PSUM"))
        o3_ps = ctx.enter_context(tc.tile_pool(name="o3_ps", bufs=2, space="PSUM"))

        qk_pool = ctx.enter_context(tc.tile_pool(name="qk", bufs=2))
        sm_pool = ctx.enter_context(tc.tile_pool(name="smp", bufs=2))
        gt_pool = ctx.enter_context(tc.tile_pool(name="gtp", bufs=2))
        out_pool = ctx.enter_context(tc.tile_pool(name="outp", bufs=2))

        # ---- HAM warm-up: open the PE clock gate before real work arrives.
        p_warm = mm_ps.tile([128, 512], dt.float32, tag="mm", name="p_warm")
        for i in range(14):
            nc.tensor.matmul(p_warm, lhsT=warm_w, rhs=warm_x,
                             start=(i == 0), stop=(i == 13))

        # ---- phase 1: fp8 DoubleRow q/k projections + scores + softmax ----
        def qk_pair(h, part, dst, l, nl):
            p_qk = mm_ps.tile([128, nl * S], dt.float32, tag="mm", name="p_qk")
            for c in range(4):
                nc.tensor.matmul(
                    p_qk,
                    lhsT=wqk8_sb[(part, h)][:, c],
                    rhs=x8_sb[c][:, :, l * S:(l + nl) * S],
                    start=(c == 0),
                    stop=(c == 3),
                    perf_mode=PM.DoubleRow,
                )
            src = p_qk.rearrange("p (a b) -> p a b", a=nl)
            nc.vector.tensor_copy(out=dst[:, l:l + nl, :], in_=src)

        def qk_proj_thunks(h):
            # alternating q/k pair thunks so early layers complete first
            qT = qk_pool.tile([128, L, S], dt.bfloat16, tag="qT", name="qT_sb")
            kT = qk_pool.tile([128, L, S], dt.bfloat16, tag="kT", name="kT_sb")
            thunks = []
            l = h
            while l < L:
                nl = 2 if l + 1 < L else 1
                for part, dst in (("q", qT), ("k", kT)):
                    thunks.append(
                        lambda part=part, dst=dst, l=l, nl=nl:
                        qk_pair(h, part, dst, l, nl))
                l += nl
            return (qT, kT), thunks

        qT0 = qk_pool.tile([128, L, S], dt.bfloat16, tag="qT", name="qT_sb")
        kT0 = qk_pool.tile([128, L, S], dt.bfloat16, tag="kT", name="kT_sb")
        for p in range(4):
            qk_pair(0, "q", qT0, 2 * p, 2)
            qk_pair(0, "k", kT0, 2 * p, 2)

        def emit_vgroup(tt, g):
            l = tt // 2
            ncols = 128 * (l + 1)
            n_g = min(512, ncols - 512 * g)
            p_v = mm_ps.tile([128, n_g], dt.float32, tag="mm", name="p_v")
            for e in range(NE):
                nc.tensor.matmul(
                    p_v,
                    lhsT=xT_sb[e][:, tt * 128:(tt + 1) * 128],
                    rhs=wv_sb[e][:, 512 * g: 512 * g + n_g],
                    start=(e == 0),
                    stop=(e == NE - 1),
                )
            nc.scalar.copy(
                out=v_slice(l, tt % 2, 512 * g, 512 * g + n_g), in_=p_v)

        # early v-proj token blocks absorbed into phase-1 head sections
        # (tt 0-7 have a single 512-col group each; xT half-0 arrives by then)
        V_EARLY = {2: [0], 3: [1, 2], 4: [3, 4], 5: [5, 6], 6: [7]}
        V_EARLY_ALL = [tt for v in V_EARLY.values() for tt in v]

        qk_tiles = (qT0, kT0)
        for h in range(7):
            qT_sb, kT_sb = qk_tiles
            if h + 1 < 7:
                qk_tiles, thunks = qk_proj_thunks(h + 1)
            else:
                thunks = []
            # interleave next head's DR pairs between this head's scores so
            # the ACT exp chain and the DVE psum-drain copies overlap.
            for li, l in enumerate(range(h, L)):
                blk = BLK[(l, h)]
                p_sc = sc_ps.tile([128, 2, S], dt.float32, tag="sc", name="p_sc")
                for tc2 in range(2):
                    nc.tensor.matmul(
                        p_sc[:, tc2, :],
                        lhsT=kT_sb[:, l, tc2 * 128:(tc2 + 1) * 128],
                        rhs=qT_sb[:, l, :],
                        start=True,
                        stop=True,
                    )
                idx = l * H + h
                nc.scalar.activation(
                    out=E_all[:, blk],
                    in_=p_sc,
                    func=AF.Exp,
                    scale=dec_sb[:, idx:idx + 1],
                )
                if li < len(thunks):
                    thunks[li]()
            for t in thunks[len(range(h, L)):]:
                t()
            # softmax denominator summed ON THE PE: identity-matmul psum
            # accumulation over the head's exp blocks (cheap 512-row bf16
            # matmuls; keeps the slow vector engines off the critical chain).
            p_D = o3_ps.tile([128, 2, S], dt.float32, tag="oy", name="p_D")
            for li, l in enumerate(range(h, L)):
                nc.tensor.matmul(
                    p_D,
                    lhsT=id_sb,
                    rhs=E_all[:, BLK[(l, h)]],
                    start=(li == 0),
                    stop=(l == L - 1),
                )
            U32 = sm_pool.tile([128, 2, S], dt.float32, tag="R", name="U32")
            nc.vector.reciprocal_approx_fast(out=U32, in_=p_D)
            nc.vector.tensor_copy(out=U_all[:, h], in_=U32)
            for tt in V_EARLY.get(h, []):
                emit_vgroup(tt, 0)

        # ---- phase 2: v projection for the remaining tts, with the
        # at = E*U multiplies interleaved on DVE (attention weights fully
        # formed before phase 3, keeping phase 3 PE-bound).
        at_muls = [(h, l) for h in range(7) for l in range(h, L)]
        rem_tts = [tt for tt in range(16) if tt not in V_EARLY_ALL]
        n_vgroups = sum((128 * (tt // 2 + 1) + 511) // 512 for tt in rem_tts)
        per_group = (len(at_muls) + n_vgroups - 1) // n_vgroups
        for tt in rem_tts:
            l = tt // 2
            ncols = 128 * (l + 1)
            for g in range((ncols + 511) // 512):
                n_g = min(512, ncols - 512 * g)
                emit_vgroup(tt, g)
                for _ in range(per_group):
                    if at_muls:
                        ah, al = at_muls.pop(0)
                        ab = BLK[(al, ah)]
                        nc.vector.tensor_mul(
                            E_all[:, ab], E_all[:, ab], U_all[:, ah])
        while at_muls:
            ah, al = at_muls.pop(0)
            ab = BLK[(al, ah)]
            nc.vector.tensor_mul(E_all[:, ab], E_all[:, ab], U_all[:, ah])

        # ---- phase 3: attention + s'-moving out-projection ----
        def emit_proj(h, gt_sb, dma_eng):
            # yT[f, s'] = woT.T @ GT_h, only the nonzero s' >= 32h columns.
            n_c = S - 32 * h
            o_sb = out_pool.tile([128, NE, S], dt.bfloat16, tag="o", name="o_sb")
            for fb in range(NE):
                p_y = o3_ps.tile([128, S], dt.float32, tag="oy", name="p_y")
                for j in range(NE):
                    nc.tensor.matmul(
                        p_y[:, :n_c],
                        lhsT=wo_sb[j][:, fb * 128:(fb + 1) * 128],
                        rhs=gt_sb[:, j, 32 * h:S],
                        start=(j == 0),
                        stop=(j == NE - 1),
                    )
                if fb % 2 == 0:
                    nc.vector.tensor_copy(out=o_sb[:, fb, :n_c], in_=p_y[:, :n_c])
                else:
                    nc.scalar.copy(out=o_sb[:, fb, :n_c], in_=p_y[:, :n_c])
            nsplit = 4 if h >= 5 else 2
            for fbh in range(nsplit):
                w_fb = NE // nsplit
                dma_eng.dma_start(
                    out=y_d[fbh * w_fb * 128:(fbh + 1) * w_fb * 128,
                            h * S + 32 * h:(h + 1) * S].rearrange(
                        "(fb p) c -> p fb c", p=128
                    ),
                    in_=o_sb[:, fbh * w_fb:(fbh + 1) * w_fb, :n_c],
                )

        # h=7 first: attn == 1 exactly (single unmasked layer) -> out2 is the
        # column sum of v[l=7, h=7]; its out-projection fills the PE at the
        # phase-3 start where head 0 has no deferred work yet.
        gt7 = gt_pool.tile([128, NE, S], dt.bfloat16, tag="gt", name="gt7_sb")
        p_o7 = sc_ps.tile([128, S], dt.float32, tag="sc", name="p_o7")
        for tc2 in range(2):
            nc.tensor.matmul(
                p_o7,
                lhsT=v_slice(7, tc2, 7 * 128, 8 * 128),
                rhs=ones_sb,
                start=(tc2 == 0),
                stop=(tc2 == 1),
            )
        nc.vector.tensor_copy(
            out=gt7[:, :, 7 * 32:8 * 32],
            in_=p_o7.rearrange("p (si j) -> p j si", j=8),
        )

        pending = (7, gt7)
        for h in range(7):
            gt_sb = gt_pool.tile([128, NE, S], dt.bfloat16, tag="gt", name="gt_sb")
            for li, l in enumerate(range(h, L)):
                blk = BLK[(l, h)]
                p_o2 = sc_ps.tile([128, S], dt.float32, tag="sc", name="p_o2")
                for tc2 in range(2):
                    nc.tensor.matmul(
                        p_o2,
                        lhsT=v_slice(l, tc2, h * 128, (h + 1) * 128),
                        rhs=E_all[:, blk, tc2, :],
                        start=(tc2 == 0),
                        stop=(tc2 == 1),
                    )
                geng = nc.vector if l % 2 == 0 else nc.scalar
                if l % 2 == 0:
                    nc.vector.tensor_copy(
                        out=gt_sb[:, :, l * 32:(l + 1) * 32],
                        in_=p_o2.rearrange("p (si j) -> p j si", j=8),
                    )
                else:
                    nc.scalar.copy(
                        out=gt_sb[:, :, l * 32:(l + 1) * 32],
                        in_=p_o2.rearrange("p (si j) -> p j si", j=8),
                    )
                if li == 1 and pending is not None:
                    emit_proj(pending[0], pending[1], nc.sync)
                    pending = None
            pending = (h, gt_sb)
        emit_proj(pending[0], pending[1], nc.scalar)

    nc.compile()
    return nc


def prepare_in_maps_fast(x, w_qkv, w_out, decay_params):
    bf16 = ml_dtypes.bfloat16
    e4 = ml_dtypes.float8_e4m3fn

    wqk = w_qkv[:2 * E]                                          # [2E, E]
    wqk8 = np.ascontiguousarray(
        (wqk.astype(np.float32) * 64.0)
        .reshape(2, H, d, NE, 128)
        .transpose(0, 1, 4, 3, 2)
    ).reshape(2, H, 128, E)[:, :7].astype(e4)
    wqk8 = np.ascontiguousarray(wqk8)
    wvT = np.ascontiguousarray(w_qkv[2 * E:].astype(bf16).T)     # [E, E]
    woutT = np.ascontiguousarray(w_out.astype(bf16).T)           # [E, E]

    ident = np.eye(128, dtype=ml_dtypes.bfloat16)
    in_maps = []
    for b in range(B):
        xTf = x[b].reshape(T, E).T                               # [E, T] fp32
        xT = np.ascontiguousarray(xTf.astype(bf16))
        x8 = np.ascontiguousarray((xTf * 16.0).astype(e4))
        dec = np.ascontiguousarray(
            np.broadcast_to(
                (decay_params[b, :L, :H] / (np.sqrt(d) * (16.0 * 64.0) ** 2))
                .astype(np.float32)
                .reshape(1, L * H),
                (128, L * H),
            )
        )
        in_maps.append({
            "x8": x8, "xT": xT, "wqk8": wqk8, "wvT": wvT,
            "woutT": woutT, "decay": dec, "ident": ident,
        })
    return in_maps


def unshard_fast(results):
    out = np.empty((B, H, S, E), dtype=np.float32)
    for b, r in enumerate(results):
        yT = np.asarray(r["y"]).astype(np.float32)               # [E, H*S]
        arr = yT.reshape(E, H, S).transpose(1, 2, 0).copy()      # [H, S, E]
        for h in range(H):
            arr[h, :32 * h, :] = 0.0
        out[b] = arr
    return out


def _build_bias(with_bias):
    import concourse.bass as bass
    import concourse.tile as tile
    import concourse.mybir as mybir
    from concourse import bacc
    from contextlib import ExitStack

    dt = mybir.dt
    AF = mybir.ActivationFunctionType

    nc = bacc.Bacc("TRN2", target_bir_lowering=False, debug=False, num_devices=8)

    xT_d = nc.dram_tensor("xT", [E, T], dt.bfloat16, kind="ExternalInput").ap()
    # q/k weights pre-packed on host as [part, head, p, e*128+m] so each
    # (part, head) loads with ONE contiguous-per-partition DMA.
    wqkp_d = nc.dram_tensor("wqkp", [2, H, 128, E], dt.bfloat16, kind="ExternalInput").ap()
    wv_d = nc.dram_tensor("wvT", [E, E], dt.bfloat16, kind="ExternalInput").ap()
    wo_d = nc.dram_tensor("woutT", [E, E], dt.bfloat16, kind="ExternalInput").ap()
    dec_d = nc.dram_tensor("decay", [128, L * H], dt.float32, kind="ExternalInput").ap()
    id_d = nc.dram_tensor("ident", [128, 128], dt.bfloat16, kind="ExternalInput").ap()
    if with_bias:
        bq_d = nc.dram_tensor("bqkv", [1, F], dt.bfloat16, kind="ExternalInput").ap()
        bo_d = nc.dram_tensor("bout", [1, E], dt.bfloat16, kind="ExternalInput").ap()
        bor_d = nc.dram_tensor("bout_row", [128, E], dt.float32, kind="ExternalInput").ap()
    y_d = nc.dram_tensor("y", [H, S, E], dt.float32, kind="ExternalOutput").ap()

    with ExitStack() as ctx:
        ctx.enter_context(
            nc.allow_low_precision(
                reason="bf16 softmax intermediates; end-to-end error ~3e-3 of scale"
            )
        )
        tc = ctx.enter_context(tile.TileContext(nc))

        consts = ctx.enter_context(tc.tile_pool(name="consts", bufs=1))
        # One tile + one DMA writer per chunk: a tile written by DMAs spread
        # over many HW queues gives its first consumer more sync waits than
        # the MM instruction can encode ("Too many sync wait commands"), and
        # fine-grained tiles let compute start as soon as its chunk lands.
        xT_sb = [consts.tile([128, T], dt.bfloat16, name=f"xT{e}") for e in range(NE)]
        wqv_sb = [consts.tile([128, E], dt.bfloat16, name=f"wqv{e}") for e in range(NE)]
        # q/k weights: one tile per (part, head) holding all e-chunks
        # side-by-side, loaded head-major so head 0's tiles land first.
        wqk_sb = {
            (part, h): consts.tile([128, E], dt.bfloat16, name=f"w{part}{h}")
            for part in ("q", "k")
            for h in range(H)
        }
        wo_sb = [consts.tile([128, E], dt.bfloat16, name=f"wo{e}") for e in range(NE)]
        dec_sb = consts.tile([128, L * H], dt.float32)
        v_sb = consts.tile([128, len(PAIRS), 2, d], dt.bfloat16)
        zrow_sb = consts.tile([128, 512], dt.float32)

        if with_bias:
            bq_sb = consts.tile([1, F], dt.bfloat16)
            bo_sb = consts.tile([1, E], dt.bfloat16)
            ones_sb = consts.tile([1, 512], dt.bfloat16)
            borow_sb = consts.tile([128, E], dt.float32)
            nc.sync.dma_start(out=bq_sb, in_=bq_d)
            nc.sync.dma_start(out=bo_sb, in_=bo_d)
            nc.sync.dma_start(out=borow_sb, in_=bor_d)
            nc.vector.memset(ones_sb, 1.0)

        nc.vector.memset(zrow_sb, 0.0)
        nc.sync.dma_start(out=dec_sb, in_=dec_d)
        # Three DMA queues in parallel, each loading in consumption order:
        #   SP:    xT chunks (feed everything, needed from the first MM)
        #   ACT:   q/k weight tiles, head-major (head 0 first)
        #   Pool:  v weights (needed ~15us in), then out-proj weights (~60us)
        # Each dma_start lands on ONE ~25-40GB/s HW ring (8 rings per issuing
        # engine). Split tensors into chunks and issue in exact consumption
        # order, ~6MB per issuing queue:
        #   SP:  xT quarter-waves (quarter q == layer-pair q), then w_out
        #   ACT: q/k weights for heads 0-1, then w_v, then remaining heads
        def _wqk_dma(part, pi, h):
            for half in range(2):
                nc.scalar.dma_start(
                    out=wqk_sb[(part, h)][:, half * 512:(half + 1) * 512],
                    in_=wqkp_d[pi, h, :, half * 512:(half + 1) * 512],
                )

        def _wqk_dma_sp(part, pi, h):
            for half in range(2):
                nc.sync.dma_start(
                    out=wqk_sb[(part, h)][:, half * 512:(half + 1) * 512],
                    in_=wqkp_d[pi, h, :, half * 512:(half + 1) * 512],
                )

        # SP row (fast): head-0 weights (first-MM gate), xT quarters, head-1
        # weights, w_out. ACT row (slower): v weights, then heads 2..7.
        for pi, part in ((0, "q"), (1, "k")):
            _wqk_dma_sp(part, pi, 0)
        for q in range(4):
            if q == 3:
                # head-1 weights land before xT's last quarter so qk_proj(1)
                # never stalls at the front->head-loop transition
                for pi, part in ((0, "q"), (1, "k")):
                    _wqk_dma_sp(part, pi, 1)
            for e in range(NE):
                nc.sync.dma_start(
                    out=xT_sb[e][:, q * 512:(q + 1) * 512],
                    in_=xT_d[e * 128:(e + 1) * 128, q * 512:(q + 1) * 512],
                )
        for e in range(NE):
            for half in range(2):
                nc.sync.dma_start(
                    out=wo_sb[e][:, half * 512:(half + 1) * 512],
                    in_=wo_d[e * 128:(e + 1) * 128, half * 512:(half + 1) * 512],
                )
        for g in range(2):
            for e in range(NE):
                nc.scalar.dma_start(
                    out=wqv_sb[e][:, g * 512:(g + 1) * 512],
                    in_=wv_d[e * 128:(e + 1) * 128, g * 512:(g + 1) * 512],
                )
        for h in range(2, H):
            for pi, part in ((0, "q"), (1, "k")):
                _wqk_dma(part, pi, h)

        mm_ps = ctx.enter_context(tc.tile_pool(name="mm_ps", bufs=3, space="PSUM"))
        sc_ps = ctx.enter_context(tc.tile_pool(name="sc_ps", bufs=3, space="PSUM"))
        o2_ps = ctx.enter_context(tc.tile_pool(name="o2_ps", bufs=2, space="PSUM"))

        qk_pool = ctx.enter_context(tc.tile_pool(name="qk", bufs=2))
        exp_pool = ctx.enter_context(tc.tile_pool(name="expp", bufs=1))
        sm_pool = ctx.enter_context(tc.tile_pool(name="smp", bufs=2))
        at_pool = ctx.enter_context(tc.tile_pool(name="atp", bufs=3))
        gt_pool = ctx.enter_context(tc.tile_pool(name="gtp", bufs=2))
        out_pool = ctx.enter_context(tc.tile_pool(name="outp", bufs=3))

        def v_proj(tts, gs=(0, 1)):
            # v projection (natural [token, dd] layout): stationary xT tile,
            # moving w columns. Only heads h <= l are ever read.
            for tt in tts:
                l = tt // 2
                ncols = 128 * (l + 1)
                for g in range((ncols + 511) // 512):
                    if g not in gs:
                        continue
                    n_g = min(512, ncols - 512 * g)
                    p_v = mm_ps.tile([128, n_g], dt.float32, tag="mm", name="p_v")
                    for e in range(NE):
                        nc.tensor.matmul(
                            p_v,
                            lhsT=xT_sb[e][:, tt * 128:(tt + 1) * 128],
                            rhs=wqv_sb[e][:, 512 * g: 512 * g + n_g],
                            start=(e == 0),
                            stop=(e == NE - 1) and not with_bias,
                        )
                    if with_bias:
                        nc.tensor.matmul(
                            p_v,
                            lhsT=ones_sb[:, :128],
                            rhs=bq_sb[:, 2 * E + 512 * g: 2 * E + 512 * g + n_g],
                            start=False,
                            stop=True,
                        )
                    for hh in range(4 * g, 4 * g + n_g // 128):
                        nc.vector.tensor_copy(
                            out=v_sb[:, BLK[(l, hh)], tt % 2, :],
                            in_=p_v[:, (hh - 4 * g) * 128:(hh - 4 * g + 1) * 128],
                        )

        # ---- per-head pipeline: q/k projection -> scores -> softmax-over-l ->
        # attn@v -> scatter into the scrambled proj input -> out projection.
        # The q/k projection for head h+1 is emitted between head h's scores
        # and attn@v so the PE has work while the softmax chain (ACT+DVE) runs.
        def qk_pair(h, part, base, dst, l, nl):
            p_qk = mm_ps.tile([128, nl * S], dt.float32, tag="mm", name="p_qk")
            for e in range(NE):
                nc.tensor.matmul(
                    p_qk,
                    lhsT=wqk_sb[(part, h)][:, e * 128:(e + 1) * 128],
                    rhs=xT_sb[e][:, l * S:(l + nl) * S],
                    start=(e == 0),
                    stop=(e == NE - 1) and not with_bias,
                )
            if with_bias:
                nc.tensor.matmul(
                    p_qk,
                    lhsT=bq_sb[:, base + h * 128: base + (h + 1) * 128],
                    rhs=ones_sb[:, :nl * S],
                    start=False,
                    stop=True,
                )
            src = p_qk.rearrange("p (a b) -> p a b", a=nl)
            nc.vector.tensor_copy(out=dst[:, l:l + nl, :], in_=src)

        def qk_proj(h):
            qT = qk_pool.tile([128, L, S], dt.bfloat16, tag="qT", name="qT_sb")
            kT = qk_pool.tile([128, L, S], dt.bfloat16, tag="kT", name="kT_sb")
            for part, base, dst in (("q", 0, qT), ("k", E, kT)):
                l = h
                while l < L:
                    nl = 2 if l + 1 < L else 1  # pair layers: N=512 moving dim
                    qk_pair(h, part, base, dst, l, nl)
                    l += nl
            return qT, kT

        # ---- HAM warm-up: dummy matmuls with no DMA deps fill the input
        # lead-in and open the PE clock gate before real work arrives.
        warm_w = consts.tile([128, 128], dt.bfloat16, name="warm_w")
        warm_x = consts.tile([128, 512], dt.bfloat16, name="warm_x")
        nc.vector.memset(warm_w, 0.0)
        nc.vector.memset(warm_x, 0.0)
        p_warm = mm_ps.tile([128, 512], dt.float32, tag="mm", name="p_warm")
        for i in range(14):
            nc.tensor.matmul(p_warm, lhsT=warm_w, rhs=warm_x,
                             start=(i == 0), stop=(i == 13))

        # ---- front: head-0 q/k pairs and v-groups interleaved by xT quarter,
        # matching DMA arrival order so the in-order PE never blocks on a
        # not-yet-loaded chunk.
        qT0 = qk_pool.tile([128, L, S], dt.bfloat16, tag="qT", name="qT_sb")
        kT0 = qk_pool.tile([128, L, S], dt.bfloat16, tag="kT", name="kT_sb")
        for p in range(4):
            qk_pair(0, "q", 0, qT0, 2 * p, 2)
            qk_pair(0, "k", E, kT0, 2 * p, 2)
            v_proj(range(4 * p, 4 * p + 4))
        def emit_proj(h, gt_sb):
            # out projection for head h: y[h, s', :] = GT.T @ woutT (+ b_out)
            for st in range(2):
                if st == 0 and h >= 4:
                    # rows s' in [0,128) are exactly zero for h >= 4
                    for ng in range(2):
                        src = borow_sb[:, ng * 512:(ng + 1) * 512] if with_bias else zrow_sb
                        nc.sync.dma_start(
                            out=y_d[h, :128, ng * 512:(ng + 1) * 512], in_=src
                        )
                    continue
                for ng in range(2):
                    p_pr = mm_ps.tile([128, 512], dt.float32, tag="mm", name="p_pr")
                    for j in range(NE):
                        nc.tensor.matmul(
                            p_pr,
                            lhsT=gt_sb[:, j, st * 128:(st + 1) * 128],
                            rhs=wo_sb[j][:, ng * 512:(ng + 1) * 512],
                            start=(j == 0),
                            stop=(j == NE - 1) and not with_bias,
                        )
                    if with_bias:
                        nc.tensor.matmul(
                            p_pr,
                            lhsT=ones_sb[:, :128],
                            rhs=bo_sb[:, ng * 512:(ng + 1) * 512],
                            start=False,
                            stop=True,
                        )
                    o_sb = out_pool.tile([128, 512], dt.float32, tag="o", name="o_sb")
                    nc.scalar.copy(out=o_sb, in_=p_pr)
                    nc.sync.dma_start(
                        out=y_d[h, st * 128:(st + 1) * 128, ng * 512:(ng + 1) * 512],
                        in_=o_sb,
                    )

        qk_tiles = (qT0, kT0)
        pending_proj = None  # (h, gt) deferred one head: extra PE gap filler
        for h in range(H):
            qT_sb, kT_sb = qk_tiles
            # scores (transposed [t, s]) + exp with decay/sqrt(d) folded into
            # the activation scale; D accumulates the softmax denominator.
            # Softmax intermediates in bf16: DVE runs 2-byte SBUF ops in the
            # fast perf modes, and the end-to-end error stays ~3e-3 of scale.
            E_sb = exp_pool.tile([128, L, 2, S], dt.bfloat16, tag="E", name="E_sb")
            D_sb = sm_pool.tile([128, 2, S], dt.bfloat16, tag="D", name="D_sb")
            for l in range(h, L):
                p_sc = sc_ps.tile([128, 2, S], dt.float32, tag="sc", name="p_sc")
                for tc2 in range(2):
                    nc.tensor.matmul(
                        p_sc[:, tc2, :],
                        lhsT=kT_sb[:, l, tc2 * 128:(tc2 + 1) * 128],
                        rhs=qT_sb[:, l, :],
                        start=True,
                        stop=True,
                    )
                idx = l * H + h
                nc.scalar.activation(
                    out=E_sb[:, l, :, :],
                    in_=p_sc,
                    func=AF.Exp,
                    scale=dec_sb[:, idx:idx + 1],
                )
                if l == h:
                    nc.vector.tensor_copy(out=D_sb, in_=E_sb[:, l, :, :])
                else:
                    nc.vector.tensor_add(D_sb, D_sb, E_sb[:, l, :, :])

            # next head's projection + previous head's out-projection fill
            # the PE while this head's softmax chain finishes on ACT/DVE
            if h + 1 < H:
                qk_tiles = qk_proj(h + 1)
            if pending_proj is not None:
                emit_proj(*pending_proj)
                pending_proj = None

            U_sb = sm_pool.tile([128, 2, S], dt.bfloat16, tag="U", name="U_sb")
            nc.vector.reciprocal(out=U_sb, in_=D_sb)

            # attn @ v (output transposed [dd, s]) and scatter into GT, the
            # transposed input of the out-projection:
            #   GT[dd, j, l*32 + si] = out2T[dd, si*8 + j]
            gt_sb = gt_pool.tile([128, L, S], dt.bfloat16, tag="gt", name="gt_sb")
            if h > 0:
                nc.vector.memset(gt_sb[:, :, :h * 32], 0.0)
            for l in range(h, L):
                at_sb = at_pool.tile([128, 2, S], dt.bfloat16, tag="at", name="at_sb")
                nc.vector.tensor_mul(at_sb, E_sb[:, l, :, :], U_sb)
                p_o2 = o2_ps.tile([128, S], dt.float32, tag="o2", name="p_o2")
                for tc2 in range(2):
                    nc.tensor.matmul(
                        p_o2,
                        lhsT=v_sb[:, BLK[(l, h)], tc2, :],
                        rhs=at_sb[:, tc2, :],
                        start=(tc2 == 0),
                        stop=(tc2 == 1),
                    )
                geng = nc.vector if l % 2 == 0 else nc.scalar
                if l % 2 == 0:
                    nc.vector.tensor_copy(
                        out=gt_sb[:, :, l * 32:(l + 1) * 32],
                        in_=p_o2.rearrange("p (si j) -> p j si", j=8),
                    )
                else:
                    nc.scalar.copy(
                        out=gt_sb[:, :, l * 32:(l + 1) * 32],
                        in_=p_o2.rearrange("p (si j) -> p j si", j=8),
                    )

            pending_proj = (h, gt_sb)
        emit_proj(*pending_proj)

    nc.compile()
    return nc


def _prepare_in_maps(x, w_qkv, b_qkv, w_out, b_out, decay_params):
    bf16 = ml_dtypes.bfloat16
    with_bias = bool(np.any(b_qkv != 0) or np.any(b_out != 0))

    wqk_bf = w_qkv[:2 * E].astype(bf16)                          # [2E, E]
    # [part, head, m, e, p] -> [part, head, p, e, m]: each (part, head) tile
    # is the stationary lhsT for all e-chunks, contiguous in DRAM.
    wqkp = np.ascontiguousarray(
        wqk_bf.reshape(2, H, d, NE, 128).transpose(0, 1, 4, 3, 2)
    ).reshape(2, H, 128, E)
    wvT = np.ascontiguousarray(w_qkv[2 * E:].astype(bf16).T)     # [E, E]
    woutT = np.ascontiguousarray(w_out.astype(bf16).T)           # [E, E]

    ident = np.eye(128, dtype=ml_dtypes.bfloat16)
    in_maps = []
    for b in range(B):
        xT = np.ascontiguousarray(
            x[b].reshape(T, E).astype(bf16).T                    # [E, T]
        )
        dec = np.ascontiguousarray(
            np.broadcast_to(
                (decay_params[b, :L, :H] / np.float32(np.sqrt(d)))
                .astype(np.float32)
                .reshape(1, L * H),
                (128, L * H),
            )
        )
        m = {"xT": xT, "wqkp": wqkp, "wvT": wvT, "woutT": woutT, "decay": dec}
        if with_bias:
            m["bqkv"] = np.ascontiguousarray(b_qkv.astype(bf16).reshape(1, F))
            m["bout"] = np.ascontiguousarray(b_out.astype(bf16).reshape(1, E))
            m["bout_row"] = np.ascontiguousarray(
                np.broadcast_to(b_out.astype(np.float32).reshape(1, E), (128, E))
            )
        in_maps.append(m)
    return with_bias, in_maps

def _get_nc(key):
    if key not in _BUILD_CACHE:
        if key == "fast":
            _BUILD_CACHE[key] = build_fast()
        else:
            _BUILD_CACHE[key] = _build_bias(True)
    return _BUILD_CACHE[key]


def _run(x, w_qkv, b_qkv, w_out, b_out, decay_params, **spmd_kwargs):
    from concourse.bass_utils import run_bass_kernel_spmd

    with_bias = bool(np.any(b_qkv != 0) or np.any(b_out != 0))
    if with_bias:
        _, in_maps = _prepare_in_maps(x, w_qkv, b_qkv, w_out, b_out, decay_params)
        nc = _get_nc("bias")
        res = run_bass_kernel_spmd(nc, in_maps, core_ids=list(range(B)), **spmd_kwargs)
        out = np.stack([np.asarray(r["y"]) for r in res.results], axis=0)
        return out.astype(np.float32, copy=False), res

    in_maps = prepare_in_maps_fast(x, w_qkv, w_out, decay_params)
    nc = _get_nc("fast")
    res = run_bass_kernel_spmd(nc, in_maps, core_ids=list(range(B)), **spmd_kwargs)
    out = unshard_fast(res.results)
    return out, res


def kernel(x, w_qkv, b_qkv, w_out, b_out, decay_params):
    out, _ = _run(
        np.asarray(x), np.asarray(w_qkv), np.asarray(b_qkv),
        np.asarray(w_out), np.asarray(b_out), np.asarray(decay_params),
    )
    return out
